# revision 7
# baseline (speedup 1.0000x reference)
"""Trainium2 Bass kernel for BottleneckAttention (patch attention).

q patches [160, 5120] from z1_hat (non-overlapping 10x4 unfold),
kv patches [5551, 5120] from z2 (overlapping unfold, Hk=91 x Wk=61),
scores = q @ kv.T / 5120, softmax over kv patches, out = attn @ kv,
folded back to [1, 128, 100, 64].

Sharding: contiguous blocks of 12 kv h-rows per core (8 x 12 = 96 >= 91;
core 7 carries 5 zero-padded h-rows whose denominator contribution the
host recomputes and subtracts exactly). Every core computes all 160 q
rows against its 732 kv patches; host combines with an all-gather
softmax: out = sum(partial_num) / (sum(partial_den) - pad_correction).

Per-core kernel (raw Bass, explicit semaphores):
  phase 1 (bf16): scores computed as implicit convolution against the
    SBUF-resident z2 slice [128c, 24h, 64w] via strided 3D access
    patterns -- the kv matrix is never materialized for this phase.
  exp on ScalarE (scale = 1/5120), row-sum denominator on VectorE.
  PE transpose of exp-scores -> e_T (cast to bf16).
  phase 2 (bf16): partial_out = e_T.T @ kv_shard with a host-built
    bf16 kv shard [128, 6, 5120] resident in SBUF.
"""

import sys

sys.path.insert(0, "/opt/trn_rl_repo")

import numpy as np
import ml_dtypes

import concourse.bass as bass
import concourse.mybir as mybir

DT = mybir.dt
AF = mybir.ActivationFunctionType

# problem geometry (hardcoded from the reference module)
KC, KH, KW = 128, 10, 4
H, W = 100, 64
NH, NW = H // KH, W // KW          # 10, 16
PQ = NH * NW                       # 160 q patches
D = KC * KH * KW                   # 5120
HK, WK = H - KH + 1, W - KW + 1    # 91, 61
NCORES = 8
HPC = 12                           # kv h-rows per core (8*12 = 96 >= 91)
PKC = HPC * WK                     # 732 kv patches per core
T = 6                              # ceil(732/128) k-chunks for phase 2
PKP = T * 128                      # 768 padded kv rows
G0H, G1H = 7, 5                    # phase-1 h-groups (7+5 = 12)
N0, N1 = G0H * WK, G1H * WK        # 427, 305
ZROWS = 2 * HPC                    # 24 z rows staged per core
SCALE = 1.0 / D

# phase-1 matmul dtype: bfloat16 runs at full PE rate and supports the
# strided 3D access patterns of the implicit convolution (fp32r rejects
# them; fp32 runs at 1/4 rate). Softmax averaging attenuates the score
# rounding noise, so end-to-end error stays ~1e-3.
P1_DT = DT.bfloat16
P1_NP = ml_dtypes.bfloat16

_CACHE = {}


def _build_nc():
    nc = bass.Bass()
    z_d = nc.declare_dram_parameter("z", [KC, ZROWS, W], P1_DT, isOutput=False)
    q_d = nc.declare_dram_parameter("qT3", [KC, KH * KW, PQ], P1_DT, isOutput=False)
    kv_d = nc.declare_dram_parameter("kvr", [128, T, D], DT.bfloat16, isOutput=False)
    out_d = nc.declare_dram_parameter("out", [PQ, D], DT.float32, isOutput=True)
    den_d = nc.declare_dram_parameter("den", [PQ, 1], DT.float32, isOutput=True)

    from contextlib import ExitStack

    ctx = ExitStack()
    with ctx:
        z_sb = ctx.enter_context(nc.sbuf_tensor([KC, ZROWS, W], P1_DT))
        q_sb = ctx.enter_context(nc.sbuf_tensor([KC, KH * KW, PQ], P1_DT))
        kv_sb = ctx.enter_context(nc.sbuf_tensor([128, T, D], DT.bfloat16))
        e_hi = ctx.enter_context(nc.sbuf_tensor([128, PKC], DT.float32))
        e_lo = ctx.enter_context(nc.sbuf_tensor([32, PKC], DT.float32))
        eT_sb = ctx.enter_context(nc.sbuf_tensor([128, T, PQ], DT.bfloat16))
        o_hi = ctx.enter_context(nc.sbuf_tensor([128, D], DT.float32))
        o_lo = ctx.enter_context(nc.sbuf_tensor([32, D], DT.float32))
        iden = ctx.enter_context(nc.sbuf_tensor([128, 128], DT.float32))
        bias0 = ctx.enter_context(nc.sbuf_tensor([128, 1], DT.float32))
        dh_sb = ctx.enter_context(nc.sbuf_tensor([128, 1], DT.float32))
        dl_sb = ctx.enter_context(nc.sbuf_tensor([32, 1], DT.float32))

        # phase-1 score accumulators: (h-group, q-half)
        ps_s = [
            ctx.enter_context(nc.psum_tensor(f"ps_s{i}", [128, n], DT.float32))
            for i, n in enumerate((N0, N0, N1, N1))
        ]  # order: g0m0, g0m1, g1m0, g1m1
        # transpose staging (4 distinct banks)
        ps_t = [
            ctx.enter_context(nc.psum_tensor(f"ps_t{i}", [128, 512], DT.float32))
            for i in range(4)
        ]

        s_z = ctx.enter_context(nc.semaphore("s_z"))
        s_q = ctx.enter_context(nc.semaphore("s_q"))
        s_kv = ctx.enter_context(nc.semaphore("s_kv"))
        s_p = ctx.enter_context(nc.semaphore("s_p"))
        s_a = ctx.enter_context(nc.semaphore("s_a"))
        s_v = ctx.enter_context(nc.semaphore("s_v"))
        s_g = ctx.enter_context(nc.semaphore("s_g"))
        s_o = ctx.enter_context(nc.semaphore("s_o"))

        # 20 phase-2 output groups (m-half x n-tile of 512)
        NT = D // 512  # 10
        groups = [(m, n) for m in range(2) for n in range(NT)]
        # e column chunks for transposes: 5 x 128 + 92
        csizes = [128] * 5 + [PKC - 5 * 128]

        with nc.Block() as block:

            @block.gpsimd
            def _(g):
                g.memset(eT_sb[:], 0.0)
                g.memset(bias0[:], 0.0)
                g.memset(iden[:], 0.0)
                g.affine_select(
                    out=iden[:],
                    in_=iden[:],
                    compare_op=mybir.AluOpType.not_equal,
                    fill=1.0,
                    base=0,
                    pattern=[[-1, 128]],
                    channel_multiplier=1,
                ).then_inc(s_g, 1)

            @block.sync
            def _(sync):
                sync.dma_start(z_sb[:], z_d[:]).then_inc(s_z, 16)
                sync.dma_start(q_sb[:, 0:20, :], q_d[:, 0:20, :]).then_inc(s_q, 16)
                sync.dma_start(q_sb[:, 20:40, :], q_d[:, 20:40, :]).then_inc(s_q, 16)
                for c in range(3):
                    sync.dma_start(
                        kv_sb[:, 2 * c : 2 * c + 2, :], kv_d[:, 2 * c : 2 * c + 2, :]
                    ).then_inc(s_kv, 16)
                sync.wait_ge(s_v, 1)
                sync.dma_start(den_d[0:128, :], dh_sb[:]).then_inc(s_o, 16)
                sync.wait_ge(s_v, 2)
                sync.dma_start(den_d[128:160, :], dl_sb[:]).then_inc(s_o, 16)
                # o_hi complete after out-copy group 9 (s_a = 17+9 = 26)
                sync.wait_ge(s_a, 26)
                sync.dma_start(out_d[0:128, :], o_hi[:]).then_inc(s_o, 16)
                sync.wait_ge(s_a, 36)
                sync.dma_start(out_d[128:160, :], o_lo[:]).then_inc(s_o, 16)
                sync.wait_ge(s_o, 64)

            @block.tensor
            def _(pe):
                pe.wait_ge(s_z, 16)
                pe.wait_ge(s_q, 16)
                # phase 1: scores[pq, pk] += q(:,ij,:).T @ z2[., h+i, w+j]
                for ij in range(KH * KW):
                    if ij == 20:
                        pe.wait_ge(s_q, 32)
                    i_, j_ = ij // KW, ij % KW
                    st, sp = ij == 0, ij == KH * KW - 1
                    rhs0 = z_sb[:, i_ : i_ + G0H, j_ : j_ + WK]
                    rhs1 = z_sb[:, G0H + i_ : HPC + i_, j_ : j_ + WK]
                    mm = nc.tensor.matmul(
                        ps_s[0][:, :], q_sb[:, ij, 0:128], rhs0, start=st, stop=sp
                    )
                    nc.tensor.matmul(
                        ps_s[1][0:32, :], q_sb[:, ij, 128:160], rhs0, start=st, stop=sp
                    )
                    nc.tensor.matmul(
                        ps_s[2][:, :], q_sb[:, ij, 0:128], rhs1, start=st, stop=sp
                    )
                    mm = nc.tensor.matmul(
                        ps_s[3][0:32, :], q_sb[:, ij, 128:160], rhs1, start=st, stop=sp
                    )
                mm.then_inc(s_p, 1)  # s_p = 1

                # transposes of exp-scores chunks -> ps_t (fp32)
                pe.wait_ge(s_a, 4)
                pe.wait_ge(s_g, 1)
                k = 0
                for t in range(T):
                    csz = csizes[t]
                    for m in range(2):
                        msz = 128 if m == 0 else 32
                        src = (
                            e_hi[:, t * 128 : t * 128 + csz]
                            if m == 0
                            else e_lo[:, t * 128 : t * 128 + csz]
                        )
                        if k >= 4:
                            pe.wait_ge(s_a, k + 1)  # copy k-4 done
                        nc.tensor.matmul(
                            ps_t[k % 4][0:csz, 0:msz],
                            src,
                            iden[0:msz, 0:msz],
                            is_transpose=True,
                            start=True,
                            stop=True,
                        ).then_inc(s_p, 1)  # s_p = 2+k
                        k += 1

                # phase 2: out[pq, d] = sum_t eT[., t, pq].T @ kv[., t, d]
                pe.wait_ge(s_a, 16)
                pe.wait_ge(s_kv, 48)
                for gidx, (m, n) in enumerate(groups):
                    m0, msz = (0, 128) if m == 0 else (128, 32)
                    if gidx >= 4:
                        pe.wait_ge(s_a, 13 + gidx)  # out-copy gidx-4 done
                    for t in range(T):
                        mm = nc.tensor.matmul(
                            ps_t[gidx % 4][0:msz, 0:512],
                            eT_sb[:, t, m0 : m0 + msz],
                            kv_sb[:, t, n * 512 : (n + 1) * 512],
                            start=(t == 0),
                            stop=(t == T - 1),
                        )
                    mm.then_inc(s_p, 1)  # s_p = 14+gidx

            @block.scalar
            def _(act):
                act.wait_ge(s_g, 1)
                act.wait_ge(s_p, 1)
                # exp(scores * SCALE) -> e
                nc.scalar.activation(
                    e_hi[:, 0:N0], ps_s[0][:, :], AF.Exp, bias=bias0[:, :], scale=SCALE
                ).then_inc(s_a, 1)
                nc.scalar.activation(
                    e_hi[:, N0:PKC], ps_s[2][:, :], AF.Exp, bias=bias0[:, :], scale=SCALE
                ).then_inc(s_a, 1)
                nc.scalar.activation(
                    e_lo[:, 0:N0],
                    ps_s[1][0:32, :],
                    AF.Exp,
                    bias=bias0[0:32, :],
                    scale=SCALE,
                ).then_inc(s_a, 1)
                nc.scalar.activation(
                    e_lo[:, N0:PKC],
                    ps_s[3][0:32, :],
                    AF.Exp,
                    bias=bias0[0:32, :],
                    scale=SCALE,
                ).then_inc(s_a, 1)  # s_a = 4
                # copy transposed chunks into eT (cast to bf16)
                k = 0
                for t in range(T):
                    csz = csizes[t]
                    for m in range(2):
                        m0, msz = (0, 128) if m == 0 else (128, 32)
                        act.wait_ge(s_p, 2 + k)
                        # centered softmax: store f = e - 1 in bf16 (|f| <~
                        # 0.08, so the cast keeps absolute precision); the
                        # host adds back the exact sum-of-kv-columns term.
                        nc.scalar.activation(
                            eT_sb[0:csz, t, m0 : m0 + msz],
                            ps_t[k % 4][0:csz, 0:msz],
                            AF.Copy,
                            bias=-1.0,
                        ).then_inc(s_a, 1)  # s_a = 5+k
                        k += 1
                # copy phase-2 accumulators to out staging
                for gidx, (m, n) in enumerate(groups):
                    msz = 128 if m == 0 else 32
                    dst = (
                        o_hi[:, n * 512 : (n + 1) * 512]
                        if m == 0
                        else o_lo[:, n * 512 : (n + 1) * 512]
                    )
                    act.wait_ge(s_p, 14 + gidx)
                    nc.scalar.activation(
                        dst, ps_t[gidx % 4][0:msz, 0:512], AF.Copy
                    ).then_inc(s_a, 1)  # s_a = 17+gidx

            @block.vector
            def _(dve):
                dve.wait_ge(s_a, 4)
                nc.vector.reduce_sum(
                    dh_sb[:], e_hi[:, :], axis=mybir.AxisListType.X
                ).then_inc(s_v, 1)
                nc.vector.reduce_sum(
                    dl_sb[:], e_lo[:, :], axis=mybir.AxisListType.X
                ).then_inc(s_v, 1)

    return nc


def _host_prep(z1_hat, z2):
    z1 = np.asarray(z1_hat, dtype=np.float32)[0]  # [128, 100, 64]
    z2a = np.asarray(z2, dtype=np.float32)[0]

    # q patches [160, 5120] and lhsT layout qT3 [128, 40, 160]
    q = z1.reshape(KC, NH, KH, NW, KW).transpose(1, 3, 0, 2, 4).reshape(PQ, D)
    qT3 = np.ascontiguousarray(
        q.reshape(PQ, KC, KH * KW).transpose(1, 2, 0), dtype=P1_NP
    )

    # padded z2: rows 100..111 zero (so out-of-range kv patches score 0)
    z_pad = np.zeros((KC, 112, W), dtype=np.float32)
    z_pad[:, :H] = z2a

    # sliding kv patches from padded z2: [96, 61, 5120]
    sw = np.lib.stride_tricks.sliding_window_view(z_pad, (KH, KW), axis=(1, 2))
    # sw: [128, 103, 61, 10, 4] -> patch(h, w) = sw[:, h, w]
    in_maps = []
    kv_pad_rows = None
    for core in range(NCORES):
        h0 = HPC * core
        blk = sw[:, h0 : h0 + HPC, :WK]  # [128, 12, 61, 10, 4]
        kv = blk.transpose(1, 2, 0, 3, 4).reshape(PKC, D)
        kvp = np.zeros((PKP, D), dtype=np.float32)
        kvp[:PKC] = kv
        # zero rows for invalid patches (h >= 91): they must not contribute
        nreal_h = max(0, min(HPC, HK - h0))
        if nreal_h < HPC:
            pad_rows = kvp[nreal_h * WK : PKC].copy()  # device-side pad patches
            kv_pad_rows = pad_rows  # used for exact denominator correction
            kvp[nreal_h * WK : PKC] = 0.0
        kvr = np.ascontiguousarray(
            kvp.reshape(T, 128, D).transpose(1, 0, 2).astype(ml_dtypes.bfloat16)
        )
        z_core = np.ascontiguousarray(z_pad[:, h0 : h0 + ZROWS, :], dtype=P1_NP)
        in_maps.append({"z": z_core, "qT3": qT3.astype(P1_NP), "kvr": kvr})

    # exact correction: core 7's pad columns contribute exp(q . pad_patch / D)
    # to its device-side denominator (pad patches built from z_pad, kv zeroed
    # on device so numerators are unaffected).
    corr = np.zeros(PQ, dtype=np.float64)
    if kv_pad_rows is not None:
        s_pad = q.astype(np.float64) @ kv_pad_rows.astype(np.float64).T  # [160, npad]
        corr = np.exp(s_pad * SCALE).sum(axis=1)
    # centered softmax: device returns f @ kv with f = e - 1; host adds the
    # exact colsum term sum_k kv[k, :] (over all real patches).
    swr = sw[:, :HK, :WK]  # [128, 91, 61, 10, 4]
    colsum = swr.astype(np.float64).sum(axis=(1, 2)).reshape(D)  # [5120]
    return in_maps, corr, colsum


def kernel(z1_hat, z2):
    from concourse.bass_utils import run_bass_kernel_spmd

    in_maps, corr, colsum = _host_prep(z1_hat, z2)
    if "nc" not in _CACHE:
        _CACHE["nc"] = _build_nc()
    nc = _CACHE["nc"]
    res = run_bass_kernel_spmd(nc, in_maps, list(range(NCORES)))
    num = np.broadcast_to(colsum, (PQ, D)).astype(np.float64).copy()
    den = -corr
    for r in res.results:
        num += r["out"].astype(np.float64)
        den = den + r["den"].astype(np.float64)[:, 0]
    out = (num / den[:, None]).astype(np.float32)
    # fold patches back: [160, 5120] -> [1, 128, 100, 64]
    out = out.reshape(NH, NW, KC, KH, KW).transpose(2, 0, 3, 1, 4)
    return np.ascontiguousarray(out.reshape(1, KC, H, W))


# revision 13
# speedup vs baseline: 1.2382x; 1.2382x over previous
"""Trainium2 Bass kernel for BottleneckAttention (patch attention).

q patches [160, 5120] from z1_hat (non-overlapping 10x4 unfold),
kv patches [5551, 5120] from z2 (overlapping unfold, Hk=91 x Wk=61),
scores = q @ kv.T / 5120, softmax over kv patches, out = attn @ kv,
folded back to [1, 128, 100, 64].

Sharding: contiguous blocks of 12 kv h-rows per core (8 x 12 = 96 >= 91;
core 7 carries 5 zero-padded h-rows whose denominator contribution the
host recomputes and subtracts exactly). Every core computes all 160 q
rows against its 732 kv patches; host combines with an all-gather
softmax: out = sum(partial_num) / (sum(partial_den) - pad_correction).

Per-core kernel (raw Bass, explicit semaphores):
  phase 1 (bf16): scores computed as implicit convolution against the
    SBUF-resident z2 slice [128c, 24h, 64w] via strided 3D access
    patterns -- the kv matrix is never materialized for this phase.
  exp on ScalarE (scale = 1/5120), row-sum denominator on VectorE.
  PE transpose of exp-scores -> e_T (cast to bf16).
  phase 2 (bf16): partial_out = e_T.T @ kv_shard with a host-built
    bf16 kv shard [128, 6, 5120] resident in SBUF.
"""

import sys

sys.path.insert(0, "/opt/trn_rl_repo")

import numpy as np
import ml_dtypes

import concourse.bass as bass
import concourse.mybir as mybir

DT = mybir.dt
AF = mybir.ActivationFunctionType

# problem geometry (hardcoded from the reference module)
KC, KH, KW = 128, 10, 4
H, W = 100, 64
NH, NW = H // KH, W // KW          # 10, 16
PQ = NH * NW                       # 160 q patches
D = KC * KH * KW                   # 5120
HK, WK = H - KH + 1, W - KW + 1    # 91, 61
NCORES = 8
HPC = 12                           # kv h-rows per core (8*12 = 96 >= 91)
PKC = HPC * WK                     # 732 kv patches per core
T = 6                              # ceil(732/128) k-chunks for phase 2
PKP = T * 128                      # 768 padded kv rows
G0H, G1H = 7, 5                    # phase-1 h-groups (7+5 = 12)
N0, N1 = G0H * WK, G1H * WK        # 427, 305
ZROWS = 2 * HPC                    # 24 z rows staged per core
SCALE = 1.0 / D

# phase-1 matmul dtype: bfloat16 runs at full PE rate and supports the
# strided 3D access patterns of the implicit convolution (fp32r rejects
# them; fp32 runs at 1/4 rate). Softmax averaging attenuates the score
# rounding noise, so end-to-end error stays ~1e-3.
P1_DT = DT.bfloat16
P1_NP = ml_dtypes.bfloat16

_CACHE = {}


def _build_nc():
    nc = bass.Bass()
    z_d = nc.declare_dram_parameter("z", [KC, ZROWS, W], P1_DT, isOutput=False)
    q_d = nc.declare_dram_parameter("qT3", [KC, KH * KW, PQ], P1_DT, isOutput=False)
    kv_d = nc.declare_dram_parameter("kvr", [128, T, D], DT.bfloat16, isOutput=False)
    out_d = nc.declare_dram_parameter("out", [PQ, D], DT.float32, isOutput=True)
    den_d = nc.declare_dram_parameter("den", [PQ, 1], DT.float32, isOutput=True)

    from contextlib import ExitStack

    ctx = ExitStack()
    with ctx:
        z_sb = ctx.enter_context(nc.sbuf_tensor([KC, ZROWS, W], P1_DT))
        q_sb = ctx.enter_context(nc.sbuf_tensor([KC, KH * KW, PQ], P1_DT))
        kv_sb = ctx.enter_context(nc.sbuf_tensor([128, T, D], DT.bfloat16))
        e_hi = ctx.enter_context(nc.sbuf_tensor([128, PKC], DT.float32))
        e_lo = ctx.enter_context(nc.sbuf_tensor([32, PKC], DT.float32))
        eT_sb = ctx.enter_context(nc.sbuf_tensor([128, T, PQ], DT.bfloat16))
        o_hi = ctx.enter_context(nc.sbuf_tensor([128, D], DT.float32))
        o_lo = ctx.enter_context(nc.sbuf_tensor([32, D], DT.float32))
        iden = ctx.enter_context(nc.sbuf_tensor([128, 128], DT.float32))
        bias0 = ctx.enter_context(nc.sbuf_tensor([128, 1], DT.float32))
        dh_sb = ctx.enter_context(nc.sbuf_tensor([128, 1], DT.float32))
        dl_sb = ctx.enter_context(nc.sbuf_tensor([32, 1], DT.float32))

        # phase-1 score accumulators: (h-group, q-half)
        ps_s = [
            ctx.enter_context(nc.psum_tensor(f"ps_s{i}", [128, n], DT.float32))
            for i, n in enumerate((N0, N0, N1, N1))
        ]  # order: g0m0, g0m1, g1m0, g1m1
        # transpose staging (4 distinct banks)
        ps_t = [
            ctx.enter_context(nc.psum_tensor(f"ps_t{i}", [128, 512], DT.float32))
            for i in range(4)
        ]

        s_z = ctx.enter_context(nc.semaphore("s_z"))
        s_qq = [ctx.enter_context(nc.semaphore(f"s_qq{i}")) for i in range(4)]
        s_kv = ctx.enter_context(nc.semaphore("s_kv"))
        s_p = ctx.enter_context(nc.semaphore("s_p"))
        s_a = ctx.enter_context(nc.semaphore("s_a"))
        s_v = ctx.enter_context(nc.semaphore("s_v"))
        s_g = ctx.enter_context(nc.semaphore("s_g"))
        s_o = ctx.enter_context(nc.semaphore("s_o"))

        # 20 phase-2 output groups (m-half x n-tile of 512)
        NT = D // 512  # 10
        groups = [(m, n) for m in range(2) for n in range(NT)]
        # e column chunks for transposes: 5 x 128 + 92
        csizes = [128] * 5 + [PKC - 5 * 128]

        with nc.Block() as block:

            @block.gpsimd
            def _(g):
                # identity + zeroed eT first: PE warmup matmuls (on the
                # zeroed bf16 eT tile) gate on s_g >= 1.
                g.memset(iden[:], 0.0)
                g.affine_select(
                    out=iden[:],
                    in_=iden[:],
                    compare_op=mybir.AluOpType.not_equal,
                    fill=1.0,
                    base=0,
                    pattern=[[-1, 128]],
                    channel_multiplier=1,
                )
                g.memset(eT_sb[:], 0.0).then_inc(s_g, 1)
                g.memset(bias0[:], 0.0).then_inc(s_g, 1)

            @block.sync
            def _(sync):
                sync.dma_start(z_sb[:], z_d[:]).then_inc(s_z, 16)
                # q in quarters, each with its own semaphore (completion
                # order across DMA queues is not guaranteed)
                for qtr in range(4):
                    sl = slice(10 * qtr, 10 * qtr + 10)
                    sync.dma_start(q_sb[:, sl, :], q_d[:, sl, :]).then_inc(
                        s_qq[qtr], 16
                    )
                for c in range(3):
                    sync.dma_start(
                        kv_sb[:, 2 * c : 2 * c + 2, :], kv_d[:, 2 * c : 2 * c + 2, :]
                    ).then_inc(s_kv, 16)
                sync.wait_ge(s_v, 1)
                sync.dma_start(den_d[0:128, :], dh_sb[:]).then_inc(s_o, 16)
                sync.wait_ge(s_v, 2)
                sync.dma_start(den_d[128:160, :], dl_sb[:]).then_inc(s_o, 16)
                # out halves pipelined behind the ACT psum->sbuf copies
                # (out-copy g bumps s_a to 17+g; m0 tiles are g 0..9)
                sync.wait_ge(s_a, 21)
                sync.dma_start(out_d[0:128, 0:2560], o_hi[:, 0:2560]).then_inc(s_o, 16)
                sync.wait_ge(s_a, 26)
                sync.dma_start(out_d[0:128, 2560:], o_hi[:, 2560:]).then_inc(s_o, 16)
                sync.wait_ge(s_a, 31)
                sync.dma_start(out_d[128:160, 0:2560], o_lo[:, 0:2560]).then_inc(
                    s_o, 16
                )
                sync.wait_ge(s_a, 36)
                sync.dma_start(out_d[128:160, 2560:], o_lo[:, 2560:]).then_inc(s_o, 16)
                sync.wait_ge(s_o, 96)

            @block.tensor
            def _(pe):
                # HAM warmup: ~32 throwaway matmuls on the zeroed bf16 eT
                # tile while the input DMAs land, so phase 1 runs at the
                # warm 2.4 GHz clock instead of the cold 1.2 GHz default.
                pe.wait_ge(s_g, 1)
                for w in range(32):
                    nc.tensor.matmul(
                        ps_t[0][0:128, 0:480],
                        eT_sb[:, 0, 0:128],
                        eT_sb[:, 0:3, 0:160],
                        start=True,
                        stop=True,
                    )
                pe.wait_ge(s_z, 16)
                pe.wait_ge(s_qq[0], 16)
                # phase 1: scores[pq, pk] += q(:,ij,:).T @ z2[., h+i, w+j]
                for ij in range(KH * KW):
                    if ij in (10, 20, 30):
                        pe.wait_ge(s_qq[ij // 10], 16)
                    i_, j_ = ij // KW, ij % KW
                    st, sp = ij == 0, ij == KH * KW - 1
                    rhs0 = z_sb[:, i_ : i_ + G0H, j_ : j_ + WK]
                    rhs1 = z_sb[:, G0H + i_ : HPC + i_, j_ : j_ + WK]
                    mm = nc.tensor.matmul(
                        ps_s[0][:, :], q_sb[:, ij, 0:128], rhs0, start=st, stop=sp
                    )
                    nc.tensor.matmul(
                        ps_s[1][0:32, :], q_sb[:, ij, 128:160], rhs0, start=st, stop=sp
                    )
                    nc.tensor.matmul(
                        ps_s[2][:, :], q_sb[:, ij, 0:128], rhs1, start=st, stop=sp
                    )
                    mm = nc.tensor.matmul(
                        ps_s[3][0:32, :], q_sb[:, ij, 128:160], rhs1, start=st, stop=sp
                    )
                mm.then_inc(s_p, 1)  # s_p = 1

                # transposes of exp-scores chunks -> ps_t (fp32)
                pe.wait_ge(s_a, 4)
                pe.wait_ge(s_g, 1)
                k = 0
                for t in range(T):
                    csz = csizes[t]
                    for m in range(2):
                        msz = 128 if m == 0 else 32
                        src = (
                            e_hi[:, t * 128 : t * 128 + csz]
                            if m == 0
                            else e_lo[:, t * 128 : t * 128 + csz]
                        )
                        if k >= 4:
                            pe.wait_ge(s_a, k + 1)  # copy k-4 done
                        nc.tensor.matmul(
                            ps_t[k % 4][0:csz, 0:msz],
                            src,
                            iden[0:msz, 0:msz],
                            is_transpose=True,
                            start=True,
                            stop=True,
                        ).then_inc(s_p, 1)  # s_p = 2+k
                        k += 1

                # phase 2: out[pq, d] = sum_t eT[., t, pq].T @ kv[., t, d]
                pe.wait_ge(s_g, 2)
                pe.wait_ge(s_a, 16)
                pe.wait_ge(s_kv, 48)
                for gidx, (m, n) in enumerate(groups):
                    m0, msz = (0, 128) if m == 0 else (128, 32)
                    if gidx >= 4:
                        pe.wait_ge(s_a, 13 + gidx)  # out-copy gidx-4 done
                    for t in range(T):
                        mm = nc.tensor.matmul(
                            ps_t[gidx % 4][0:msz, 0:512],
                            eT_sb[:, t, m0 : m0 + msz],
                            kv_sb[:, t, n * 512 : (n + 1) * 512],
                            start=(t == 0),
                            stop=(t == T - 1),
                        )
                    mm.then_inc(s_p, 1)  # s_p = 14+gidx

            @block.scalar
            def _(act):
                act.wait_ge(s_g, 2)
                act.wait_ge(s_p, 1)
                # exp(scores * SCALE) -> e
                nc.scalar.activation(
                    e_hi[:, 0:N0], ps_s[0][:, :], AF.Exp, bias=bias0[:, :], scale=SCALE
                ).then_inc(s_a, 1)
                nc.scalar.activation(
                    e_hi[:, N0:PKC], ps_s[2][:, :], AF.Exp, bias=bias0[:, :], scale=SCALE
                ).then_inc(s_a, 1)
                nc.scalar.activation(
                    e_lo[:, 0:N0],
                    ps_s[1][0:32, :],
                    AF.Exp,
                    bias=bias0[0:32, :],
                    scale=SCALE,
                ).then_inc(s_a, 1)
                nc.scalar.activation(
                    e_lo[:, N0:PKC],
                    ps_s[3][0:32, :],
                    AF.Exp,
                    bias=bias0[0:32, :],
                    scale=SCALE,
                ).then_inc(s_a, 1)  # s_a = 4
                # copy transposed chunks into eT (cast to bf16)
                k = 0
                for t in range(T):
                    csz = csizes[t]
                    for m in range(2):
                        m0, msz = (0, 128) if m == 0 else (128, 32)
                        act.wait_ge(s_p, 2 + k)
                        # centered softmax: store f = e - 1 in bf16 (|f| <~
                        # 0.08, so the cast keeps absolute precision); the
                        # host adds back the exact sum-of-kv-columns term.
                        nc.scalar.activation(
                            eT_sb[0:csz, t, m0 : m0 + msz],
                            ps_t[k % 4][0:csz, 0:msz],
                            AF.Copy,
                            bias=-1.0,
                        ).then_inc(s_a, 1)  # s_a = 5+k
                        k += 1
                # copy phase-2 accumulators to out staging
                for gidx, (m, n) in enumerate(groups):
                    msz = 128 if m == 0 else 32
                    dst = (
                        o_hi[:, n * 512 : (n + 1) * 512]
                        if m == 0
                        else o_lo[:, n * 512 : (n + 1) * 512]
                    )
                    act.wait_ge(s_p, 14 + gidx)
                    nc.scalar.activation(
                        dst, ps_t[gidx % 4][0:msz, 0:512], AF.Copy
                    ).then_inc(s_a, 1)  # s_a = 17+gidx

            @block.vector
            def _(dve):
                dve.wait_ge(s_a, 4)
                nc.vector.reduce_sum(
                    dh_sb[:], e_hi[:, :], axis=mybir.AxisListType.X
                ).then_inc(s_v, 1)
                nc.vector.reduce_sum(
                    dl_sb[:], e_lo[:, :], axis=mybir.AxisListType.X
                ).then_inc(s_v, 1)

    return nc


def _host_prep(z1_hat, z2):
    z1 = np.asarray(z1_hat, dtype=np.float32)[0]  # [128, 100, 64]
    z2a = np.asarray(z2, dtype=np.float32)[0]

    # q patches [160, 5120] and lhsT layout qT3 [128, 40, 160]
    q = z1.reshape(KC, NH, KH, NW, KW).transpose(1, 3, 0, 2, 4).reshape(PQ, D)
    qT3 = np.ascontiguousarray(
        q.reshape(PQ, KC, KH * KW).transpose(1, 2, 0), dtype=P1_NP
    )

    # padded z2: rows 100..111 zero (so out-of-range kv patches score 0)
    z_pad = np.zeros((KC, 112, W), dtype=np.float32)
    z_pad[:, :H] = z2a

    # sliding kv patches from padded z2: [96, 61, 5120]
    sw = np.lib.stride_tricks.sliding_window_view(z_pad, (KH, KW), axis=(1, 2))
    # sw: [128, 103, 61, 10, 4] -> patch(h, w) = sw[:, h, w]
    in_maps = []
    kv_pad_rows = None
    for core in range(NCORES):
        h0 = HPC * core
        blk = sw[:, h0 : h0 + HPC, :WK]  # [128, 12, 61, 10, 4]
        kv = blk.transpose(1, 2, 0, 3, 4).reshape(PKC, D)
        kvp = np.zeros((PKP, D), dtype=np.float32)
        kvp[:PKC] = kv
        # zero rows for invalid patches (h >= 91): they must not contribute
        nreal_h = max(0, min(HPC, HK - h0))
        if nreal_h < HPC:
            pad_rows = kvp[nreal_h * WK : PKC].copy()  # device-side pad patches
            kv_pad_rows = pad_rows  # used for exact denominator correction
            kvp[nreal_h * WK : PKC] = 0.0
        kvr = np.ascontiguousarray(
            kvp.reshape(T, 128, D).transpose(1, 0, 2).astype(ml_dtypes.bfloat16)
        )
        z_core = np.ascontiguousarray(z_pad[:, h0 : h0 + ZROWS, :], dtype=P1_NP)
        in_maps.append({"z": z_core, "qT3": qT3.astype(P1_NP), "kvr": kvr})

    # exact correction: core 7's pad columns contribute exp(q . pad_patch / D)
    # to its device-side denominator (pad patches built from z_pad, kv zeroed
    # on device so numerators are unaffected).
    corr = np.zeros(PQ, dtype=np.float64)
    if kv_pad_rows is not None:
        s_pad = q.astype(np.float64) @ kv_pad_rows.astype(np.float64).T  # [160, npad]
        corr = np.exp(s_pad * SCALE).sum(axis=1)
    # centered softmax: device returns f @ kv with f = e - 1; host adds the
    # exact colsum term sum_k kv[k, :] (over all real patches).
    swr = sw[:, :HK, :WK]  # [128, 91, 61, 10, 4]
    colsum = swr.astype(np.float64).sum(axis=(1, 2)).reshape(D)  # [5120]
    return in_maps, corr, colsum


def kernel(z1_hat, z2):
    from concourse.bass_utils import run_bass_kernel_spmd

    in_maps, corr, colsum = _host_prep(z1_hat, z2)
    if "nc" not in _CACHE:
        _CACHE["nc"] = _build_nc()
    nc = _CACHE["nc"]
    res = run_bass_kernel_spmd(nc, in_maps, list(range(NCORES)))
    num = np.broadcast_to(colsum, (PQ, D)).astype(np.float64).copy()
    den = -corr
    for r in res.results:
        num += r["out"].astype(np.float64)
        den = den + r["den"].astype(np.float64)[:, 0]
    out = (num / den[:, None]).astype(np.float32)
    # fold patches back: [160, 5120] -> [1, 128, 100, 64]
    out = out.reshape(NH, NW, KC, KH, KW).transpose(2, 0, 3, 1, 4)
    return np.ascontiguousarray(out.reshape(1, KC, H, W))
